# revision 1
# baseline (speedup 1.0000x reference)
"""Trainium2 Bass kernel for nn_DistanceFieldPenetrationLoss.

Computes loss = sum(relu(1e-3 - tridist(A,B))) / count over 2M close pairs,
sharded data-parallel over 8 NeuronCores. Per-pair triangle rows are
pre-gathered on the host (HW indirect-DMA gathers are one-index-per-
partition on TRN2, making on-device gathers descriptor-bound) and streamed
to SBUF as contiguous DMA; all geometry runs on-device in a term-blocked
SoA layout (strided/broadcast AP views over the gathered tile, 1 instr per
blocked group of the 15 distance terms).

The per-pair triangle "distance" replicates the reference exactly:
min over 15 terms: 6 point-(column-)triangle distances + 9 row-edge/edge
distances. Point-triangle = {face-masked, 3 point-edge}; edge-edge is
evaluated in its exact boundary form: min(interior-masked, 4 point-edge),
which equals the reference's clamp/recompute algorithm mathematically.
"""
import numpy as np

import concourse.bass as bass
import concourse.bacc as bacc
import concourse.mybir as mybir
import concourse.tile as tile
from concourse.bass_utils import run_bass_kernel_spmd

F32 = mybir.dt.float32
I32 = mybir.dt.int32
Alu = mybir.AluOpType
Act = mybir.ActivationFunctionType

P = 128
B, F, PPB = 4, 50000, 500000
NPAIR = B * PPB
NCORE = 8
PER_CORE = NPAIR // NCORE          # 250000
NCOL = 1954                        # 128*1954 = 250112 slots per core
CAP = P * NCOL
import os
if os.environ.get("K_DEBUG_SMALL"):
    NCOL = 8
    CAP = P * NCOL
    TILE_W = [8]
else:
    TILE_W = [152] * 12 + [130]        # sum = 1954
EPS = 1e-12
LOSS_EPS = 1e-3
BIG = 1e30

_CACHE = {}


def _mk(ap, off, dims):
    """View into an SBUF tile AP with explicit free dims [[step, count], ...]."""
    return bass.AP(ap.tensor, ap.offset + off, [list(ap.ap[0])] + [list(d) for d in dims])


def _build_kernel(ncol=None, tile_w=None):
    NCOL = ncol if ncol is not None else globals()["NCOL"]
    TILE_W = tile_w if tile_w is not None else globals()["TILE_W"]
    nc = bacc.Bacc("TRN2", target_bir_lowering=False, debug=False)
    gdata = nc.declare_dram_parameter("gdata", [P, 18 * NCOL], F32, isOutput=False)
    maskin = nc.declare_dram_parameter("maskin", [P, NCOL], F32, isOutput=False)
    psum_out = nc.declare_dram_parameter("psum", [P, len(TILE_W)], F32, isOutput=True)
    DEBUG = bool(os.environ.get("K_DEBUG_SMALL"))
    if DEBUG:
        dist_out = nc.declare_dram_parameter("dist", [P, NCOL], F32, isOutput=True)
        accpt_out = nc.declare_dram_parameter("accpt", [P, NCOL], F32, isOutput=True)

    with tile.TileContext(nc) as tc:
        with (
            tc.tile_pool(name="gio", bufs=2) as gio,
            tc.tile_pool(name="work", bufs=1) as work,
        ):
            V = nc.vector
            S = nc.scalar

            psum_t = work.tile([P, len(TILE_W)], F32, tag="psum", name="psum")
            V.memset(psum_t[:], 0.0)

            colbase = 0
            for ti, W in enumerate(TILE_W):
                G = gio.tile([P, 18 * W], F32, tag="g", name="g")
                nc.sync.dma_start(out=G[:], in_=gdata[:, 18 * colbase:18 * (colbase + W)])
                M = gio.tile([P, W], F32, tag="mask", name="mask")
                nc.sync.dma_start(out=M[:], in_=maskin[:, colbase:colbase + W])

                Gap = G[:]

                # --- G views.  free index = w*18 + t*9 + m;  m = 3*row + col(coord)
                # row-vertex i, comp c of side t:   m = 3i + c
                # col-vertex k, comp c of side t:   m = 3c + k
                def Rblk(t, c):          # [3(vert i), W]
                    return _mk(Gap, 9 * t + c, [[3, 3], [18, W]])

                def R9A(c):              # [3(i), 3(rep j), W]
                    return _mk(Gap, c, [[3, 3], [0, 3], [18, W]])

                def R9B(c):              # [3(rep i), 3(j), W]
                    return _mk(Gap, 9 + c, [[0, 3], [3, 3], [18, W]])

                def Cv(t, c, k):         # [W] single col-vertex comp
                    return _mk(Gap, 9 * t + 3 * c + k, [[18, W]])

                def Cb3(t, c, k):        # [3(rep), W]
                    return _mk(Gap, 9 * t + 3 * c + k, [[0, 3], [18, W]])

                # tile allocation helpers (plain + shaped views)
                def TW(tag):
                    return work.tile([P, W], F32, tag=tag, name=tag)[:]

                def T3(tag):
                    return work.tile([P, 3 * W], F32, tag=tag, name=tag)[:]

                def T9(tag):
                    return work.tile([P, 9 * W], F32, tag=tag, name=tag)[:]

                def s3(ap):              # [3, W] view of 3W tile
                    return _mk(ap, 0, [[W, 3], [1, W]])

                def s9(ap):              # [3, 3, W] view of 9W tile
                    return _mk(ap, 0, [[3 * W, 3], [W, 3], [1, W]])

                def b3(ap_w):            # broadcast [W] tile over 3 blocks
                    return _mk(ap_w, 0, [[0, 3], [1, W]])

                def repA(ap3):           # [3W] tile -> [3(i), 3(rep), W]
                    return _mk(ap3, 0, [[W, 3], [0, 3], [1, W]])

                def repB(ap3):           # [3W] tile -> [3(rep), 3(j), W]
                    return _mk(ap3, 0, [[0, 3], [W, 3], [1, W]])

                def blkof(ap3, i):       # i-th W block of a 3W tile
                    return _mk(ap3, i * W, [[1, W]])

                GP = nc.gpsimd

                def tt(out, a, b, op, eng=None):
                    (eng or V).tensor_tensor(out=out, in0=a, in1=b, op=op)

                def dot3g(out, av, bv, tmp):
                    GP.tensor_tensor(out=tmp, in0=av[0], in1=bv[0], op=Alu.mult)
                    GP.tensor_tensor(out=out, in0=av[1], in1=bv[1], op=Alu.mult)
                    GP.tensor_tensor(out=out, in0=out, in1=tmp, op=Alu.add)
                    GP.tensor_tensor(out=tmp, in0=av[2], in1=bv[2], op=Alu.mult)
                    GP.tensor_tensor(out=out, in0=out, in1=tmp, op=Alu.add)

                def dot3(out, av, bv, tmp):
                    tt(tmp, av[0], bv[0], Alu.mult)
                    tt(out, av[1], bv[1], Alu.mult)
                    tt(out, out, tmp, Alu.add)
                    tt(tmp, av[2], bv[2], Alu.mult)
                    tt(out, out, tmp, Alu.add)

                def norm2(out, av, tmp):
                    S.activation(out=tmp, in_=av[0], func=Act.Square)
                    S.activation(out=out, in_=av[1], func=Act.Square)
                    tt(out, out, tmp, Alu.add)
                    S.activation(out=tmp, in_=av[2], func=Act.Square)
                    tt(out, out, tmp, Alu.add)

                def recip(out, x, tmp):
                    S.activation(out=tmp, in_=x, func=Act.Ln)
                    S.activation(out=out, in_=tmp, func=Act.Exp, scale=-1.0)

                def clip01(x):
                    V.tensor_scalar(out=x, in0=x, scalar1=0.0, scalar2=1.0,
                                    op0=Alu.max, op1=Alu.min)

                acc = TW("acc")
                V.memset(acc, BIG)

                def foldmin(blocked_ap_tile, nblk):
                    # min of nblk W-blocks of a tile into acc
                    n = nblk
                    while n > 1:
                        h = n // 2
                        lo = _mk(blocked_ap_tile, 0, [[1, h * W]])
                        hi = _mk(blocked_ap_tile, (n - h) * W, [[1, h * W]])
                        tt(lo, lo, hi, Alu.min)
                        n = n - h
                    tt(acc, acc, _mk(blocked_ap_tile, 0, [[1, W]]), Alu.min)

                # ---------- per-side derived data ----------
                side = []
                for t in (0, 1):
                    sd = {}
                    # column-triangle data
                    eC = []
                    for pair_kk in ((1, 0), (2, 0), (2, 1)):
                        comps = []
                        for c in range(3):
                            e = TW(f"eC{len(eC)}{c}_{t}")
                            tt(e, Cv(t, c, pair_kk[0]), Cv(t, c, pair_kk[1]), Alu.subtract)
                            comps.append(e)
                        eC.append(comps)
                    sd["eC"] = eC
                    tmpw = TW(f"tmpw{t}")
                    aC = TW(f"aC{t}"); norm2(aC, eC[0], tmpw)
                    V.tensor_scalar(out=aC, in0=aC, scalar1=EPS, scalar2=None, op0=Alu.max)
                    bC = TW(f"bC{t}"); dot3(bC, eC[0], eC[1], tmpw)
                    cC = TW(f"cC{t}"); norm2(cC, eC[1], tmpw)
                    V.tensor_scalar(out=cC, in0=cC, scalar1=EPS, scalar2=None, op0=Alu.max)
                    a2C = TW(f"a2C{t}"); norm2(a2C, eC[2], tmpw)
                    V.tensor_scalar(out=a2C, in0=a2C, scalar1=EPS, scalar2=None, op0=Alu.max)
                    det = TW(f"det{t}")
                    S.activation(out=tmpw, in_=bC, func=Act.Square)
                    tt(det, aC, cC, Alu.mult)
                    tt(det, det, tmpw, Alu.subtract)
                    V.tensor_scalar(out=det, in0=det, scalar1=EPS, scalar2=None, op0=Alu.max)
                    for nm, src in (("invdet", det), ("invaC", aC), ("invcC", cC), ("inva2C", a2C)):
                        dst = TW(nm + str(t)); recip(dst, src, tmpw)
                        sd[nm] = dst
                    sd.update(aC=aC, bC=bC, cC=cC, a2C=a2C, det=det)

                    # row-edge data (3W blocked: edges [R1-R0, R2-R1, R0-R2])
                    E = []
                    for c in range(3):
                        e = T3(f"E{c}_{t}")
                        # blocks 0..1: R_{i+1} - R_i
                        nxt = _mk(Gap, 9 * t + c + 3, [[3, 2], [18, W]])
                        cur = _mk(Gap, 9 * t + c, [[3, 2], [18, W]])
                        tt(_mk(e, 0, [[W, 2], [1, W]]), nxt, cur, Alu.subtract)
                        # block 2: R0 - R2
                        tt(blkof(e, 2), _mk(Gap, 9 * t + c, [[18, W]]),
                           _mk(Gap, 9 * t + c + 6, [[18, W]]), Alu.subtract)
                        E.append(e)
                    sd["E"] = E
                    tmp3 = T3(f"tmp3_{t}")
                    aE = T3(f"aE{t}")
                    norm2(aE, E, tmp3)
                    invE = T3(f"invE{t}"); recip(invE, aE, tmp3)
                    ninvE = T3(f"ninvE{t}")
                    V.tensor_scalar(out=ninvE, in0=invE, scalar1=-1.0, scalar2=None, op0=Alu.mult)
                    sd.update(aE=aE, invE=invE, ninvE=ninvE)
                    side.append(sd)

                # ---------- point-triangle, 2 directions x 3 points (3W blocked) ----------
                t3a = T3("t3a"); t3b = T3("t3b"); t3c = T3("t3c")
                for tp, tt_ in ((0, 1), (1, 0)):
                    sd = side[tt_]
                    w = [T3(f"w{c}") for c in range(3)]
                    for c in range(3):
                        tt(s3(w[c]), Rblk(tp, c), Cb3(tt_, c, 0), Alu.subtract)
                    d = T3("ptd"); dot3(d, [b3(x) for x in sd["eC"][0]], w, t3a)
                    e = T3("pte"); dot3(e, [b3(x) for x in sd["eC"][1]], w, t3a)
                    f = T3("ptf"); norm2(f, w, t3a)
                    s = T3("pts")
                    tt(t3a, b3(sd["bC"]), e, Alu.mult)
                    tt(t3b, b3(sd["cC"]), d, Alu.mult)
                    tt(s, t3a, t3b, Alu.subtract)
                    t = T3("ptt")
                    tt(t3a, b3(sd["bC"]), d, Alu.mult)
                    tt(t3b, b3(sd["aC"]), e, Alu.mult)
                    tt(t, t3a, t3b, Alu.subtract)
                    # in-face margin m = min(s, t, det-(s+t))
                    m = T3("ptm")
                    tt(m, s, t, Alu.min)
                    tt(t3a, s, t, Alu.add)
                    V.scalar_tensor_tensor(out=t3b, in0=t3a, scalar=-1.0, in1=b3(sd["det"]),
                                           op0=Alu.mult, op1=Alu.add)
                    tt(m, m, t3b, Alu.min)
                    # face distance
                    fc = T3("ptfc")
                    tt(t3a, d, s, Alu.mult)
                    tt(t3b, e, t, Alu.mult)
                    tt(t3a, t3a, t3b, Alu.add)
                    tt(t3b, f, b3(sd["det"]), Alu.mult)
                    tt(t3a, t3b, t3a, Alu.subtract)
                    tt(fc, t3a, b3(sd["invdet"]), Alu.mult)
                    V.tensor_scalar(out=fc, in0=fc, scalar1=0.0, scalar2=None, op0=Alu.max)
                    V.tensor_scalar(out=t3a, in0=m, scalar1=0.0, scalar2=BIG,
                                    op0=Alu.is_lt, op1=Alu.mult)
                    tt(fc, fc, t3a, Alu.add)
                    foldmin(fc, 3)
                    # pe01 / pe02: foot on eC0 (param d/aC) and eC1 (param e/cC)
                    for dotv, inv, ev in ((d, "invaC", 0), (e, "invcC", 1)):
                        u = t3c
                        tt(u, dotv, b3(sd[inv]), Alu.mult)
                        clip01(u)
                        pe = T3("ptpe")
                        for c in range(3):
                            tt(t3a, u, b3(sd["eC"][ev][c]), Alu.mult)
                            tt(w2c := t3b, w[c], t3a, Alu.subtract)
                            if c == 0:
                                S.activation(out=pe, in_=w2c, func=Act.Square)
                            else:
                                S.activation(out=t3a, in_=w2c, func=Act.Square)
                                tt(pe, pe, t3a, Alu.add)
                        foldmin(pe, 3)
                    # pe12: w2 = p - C1, edge eC2, param dd/a2C
                    w2 = [T3(f"w2{c}") for c in range(3)]
                    for c in range(3):
                        tt(s3(w2[c]), Rblk(tp, c), Cb3(tt_, c, 1), Alu.subtract)
                    dd = T3("ptdd"); dot3(dd, [b3(x) for x in sd["eC"][2]], w2, t3a)
                    u = t3c
                    tt(u, dd, b3(sd["inva2C"]), Alu.mult)
                    clip01(u)
                    pe = T3("ptpe")
                    for c in range(3):
                        tt(t3a, u, b3(sd["eC"][2][c]), Alu.mult)
                        tt(t3b, w2[c], t3a, Alu.subtract)
                        if c == 0:
                            S.activation(out=pe, in_=t3b, func=Act.Square)
                        else:
                            S.activation(out=t3a, in_=t3b, func=Act.Square)
                            tt(pe, pe, t3a, Alu.add)
                    foldmin(pe, 3)

                if DEBUG:
                    nc.sync.dma_start(out=accpt_out[:, colbase:colbase + W], in_=acc)
                # ---------- edge-edge, 9-blocked [3(i:A-edge), 3(j:B-edge), W] ----------
                EA, EB = side[0]["E"], side[1]["E"]
                t9a = T9("t9a"); t9b = T9("t9b"); t9c = T9("t9c")
                r = [T9(f"r{c}") for c in range(3)]
                for c in range(3):
                    tt(s9(r[c]), R9A(c), R9B(c), Alu.subtract)
                d1v = [repA(EA[c]) for c in range(3)]
                d2v = [repB(EB[c]) for c in range(3)]
                cdot = T9("cdot"); dot3(cdot, d1v, r, t9a)
                fdot = T9("fdot"); dot3(fdot, d2v, r, t9a)
                bq = T9("bq"); dot3(bq, d1v, d2v, t9a)
                aA_r = repA(side[0]["aE"]); aB_t = repB(side[1]["aE"])
                den = T9("den")
                tt(den, aA_r, aB_t, Alu.mult)
                S.activation(out=t9a, in_=bq, func=Act.Square)
                tt(den, den, t9a, Alu.subtract)
                dadj = T9("dadj")
                V.tensor_scalar(out=dadj, in0=den, scalar1=EPS, scalar2=None, op0=Alu.max)
                invd = T9("invd")
                S.activation(out=t9a, in_=dadj, func=Act.Ln)
                S.activation(out=invd, in_=t9a, func=Act.Exp, scale=-1.0)
                s = T9("ees")
                tt(t9a, bq, fdot, Alu.mult)
                tt(t9b, cdot, aB_t, Alu.mult)
                tt(s, t9a, t9b, Alu.subtract)
                tt(s, s, invd, Alu.mult)
                t = T9("eet")
                tt(t9a, aA_r, fdot, Alu.mult)
                tt(t9b, bq, cdot, Alu.mult)
                tt(t, t9a, t9b, Alu.subtract)
                tt(t, t, invd, Alu.mult)
                # Clamp s,t to [0,1]: identical when the interior solution is
                # valid; otherwise the clamped point-pair distance upper-bounds
                # the true segment distance, which the boundary point-edge
                # terms below already realize — the 15-term min is unchanged.
                clip01(s)
                clip01(t)
                d2i = T9("d2i")
                for c in range(3):
                    tt(t9a, s, d1v[c], Alu.mult)
                    tt(t9a, r[c], t9a, Alu.add)
                    tt(t9b, t, d2v[c], Alu.mult)
                    tt(t9a, t9a, t9b, Alu.subtract)
                    if c == 0:
                        S.activation(out=d2i, in_=t9a, func=Act.Square)
                    else:
                        S.activation(out=t9b, in_=t9a, func=Act.Square)
                        tt(d2i, d2i, t9b, Alu.add)
                foldmin(d2i, 9)
                # pe(A_i, B-edge j): u = clip(fdot * invE_B); v = r - u*d2
                u = t9c
                tt(u, fdot, repB(side[1]["invE"]), Alu.mult)
                clip01(u)
                pe9 = T9("pe9")
                for c in range(3):
                    tt(t9a, u, d2v[c], Alu.mult)
                    tt(t9a, r[c], t9a, Alu.subtract)
                    if c == 0:
                        S.activation(out=pe9, in_=t9a, func=Act.Square)
                    else:
                        S.activation(out=t9b, in_=t9a, func=Act.Square)
                        tt(pe9, pe9, t9b, Alu.add)
                foldmin(pe9, 9)
                # pe(B_j, A-edge i): u = clip(cdot * -invE_A); v = r + u*d1
                tt(u, cdot, repA(side[0]["ninvE"]), Alu.mult)
                clip01(u)
                for c in range(3):
                    tt(t9a, u, d1v[c], Alu.mult)
                    tt(t9a, r[c], t9a, Alu.add)
                    if c == 0:
                        S.activation(out=pe9, in_=t9a, func=Act.Square)
                    else:
                        S.activation(out=t9b, in_=t9a, func=Act.Square)
                        tt(pe9, pe9, t9b, Alu.add)
                foldmin(pe9, 9)

                # ---------- pen = relu(1e-3 - sqrt(acc)); masked accumulate ----------
                dist = TW("dist")
                S.activation(out=dist, in_=acc, func=Act.Sqrt)
                pen = TW("pen")
                V.tensor_scalar(out=pen, in0=dist, scalar1=-1.0, scalar2=LOSS_EPS,
                                op0=Alu.mult, op1=Alu.add)
                if DEBUG:
                    nc.sync.dma_start(out=dist_out[:, colbase:colbase + W], in_=dist)
                penm = TW("penm")
                V.scalar_tensor_tensor(out=penm, in0=pen, scalar=0.0, in1=M[:],
                                       op0=Alu.max, op1=Alu.mult,
                                       accum_out=psum_t[:, ti:ti + 1])
                colbase += W

            nc.sync.dma_start(out=psum_out[:], in_=psum_t[:])
    nc.compile()
    return nc




M2 = 4e-6        # (2e-3)^2 certified prune margin squared
N2MIN = 1e-4     # min |cross|^2 for a trustworthy normal direction

NCOL1 = 1954
TILE_W1 = [128] * 15 + [34]
CAP2_COL = 128                     # phase-2 slots per core = 128*128 = 16384
CAP2 = P * CAP2_COL


def _build_flags():
    """Phase-1: certified lower-bound prune. Writes per-slot flag:
    1.0 = certifiably all 15 terms > 1e-3 (pen == 0), 0.0 = needs phase 2."""
    NCOL = NCOL1
    TILE_W = TILE_W1
    nc = bacc.Bacc("TRN2", target_bir_lowering=False, debug=False)
    gdata = nc.declare_dram_parameter("gdata", [P, 18 * NCOL], F32, isOutput=False)
    flags_out = nc.declare_dram_parameter("flags", [P, NCOL], F32, isOutput=True)

    with tile.TileContext(nc) as tc:
        with (
            tc.tile_pool(name="gio", bufs=2) as gio,
            tc.tile_pool(name="work", bufs=1) as work,
        ):
            V = nc.vector
            S = nc.scalar
            colbase = 0
            for ti, W in enumerate(TILE_W):
                G = gio.tile([P, 18 * W], F32, tag="g", name="g")
                nc.sync.dma_start(out=G[:], in_=gdata[:, 18 * colbase:18 * (colbase + W)])
                Gap = G[:]

                def Rblk(t, c):
                    return _mk(Gap, 9 * t + c, [[3, 3], [18, W]])

                def R9A(c):
                    return _mk(Gap, c, [[3, 3], [0, 3], [18, W]])

                def R9B(c):
                    return _mk(Gap, 9 + c, [[0, 3], [3, 3], [18, W]])

                def Cv(t, c, k):
                    return _mk(Gap, 9 * t + 3 * c + k, [[18, W]])

                def Cb3(t, c, k):
                    return _mk(Gap, 9 * t + 3 * c + k, [[0, 3], [18, W]])

                def TW(tag):
                    return work.tile([P, W], F32, tag=tag, name=tag)[:]

                def T3(tag):
                    return work.tile([P, 3 * W], F32, tag=tag, name=tag)[:]

                def T9(tag):
                    return work.tile([P, 9 * W], F32, tag=tag, name=tag)[:]

                def s3(ap):
                    return _mk(ap, 0, [[W, 3], [1, W]])

                def s9(ap):
                    return _mk(ap, 0, [[3 * W, 3], [W, 3], [1, W]])

                def b3(ap_w):
                    return _mk(ap_w, 0, [[0, 3], [1, W]])

                def repA(ap3):
                    return _mk(ap3, 0, [[W, 3], [0, 3], [1, W]])

                def repB(ap3):
                    return _mk(ap3, 0, [[0, 3], [W, 3], [1, W]])

                def blkof(ap3, i):
                    return _mk(ap3, i * W, [[1, W]])

                def tt(out, a, b, op):
                    V.tensor_tensor(out=out, in0=a, in1=b, op=op)

                def dot3(out, av, bv, tmp):
                    tt(tmp, av[0], bv[0], Alu.mult)
                    tt(out, av[1], bv[1], Alu.mult)
                    tt(out, out, tmp, Alu.add)
                    tt(tmp, av[2], bv[2], Alu.mult)
                    tt(out, out, tmp, Alu.add)

                def norm2(out, av, tmp):
                    S.activation(out=tmp, in_=av[0], func=Act.Square)
                    S.activation(out=out, in_=av[1], func=Act.Square)
                    tt(out, out, tmp, Alu.add)
                    S.activation(out=tmp, in_=av[2], func=Act.Square)
                    tt(out, out, tmp, Alu.add)

                def fold_and(tile_ap, nblk, target):
                    n = nblk
                    while n > 1:
                        h = n // 2
                        lo = _mk(tile_ap, 0, [[1, h * W]])
                        hi = _mk(tile_ap, (n - h) * W, [[1, h * W]])
                        tt(lo, lo, hi, Alu.min)
                        n = n - h
                    if target is not None:
                        tt(target, target, _mk(tile_ap, 0, [[1, W]]), Alu.min)

                # row edges, both sides
                E = {}
                for t in (0, 1):
                    for c in range(3):
                        e = T3(f"E{c}_{t}")
                        nxt = _mk(Gap, 9 * t + c + 3, [[3, 2], [18, W]])
                        cur = _mk(Gap, 9 * t + c, [[3, 2], [18, W]])
                        tt(_mk(e, 0, [[W, 2], [1, W]]), nxt, cur, Alu.subtract)
                        tt(blkof(e, 2), _mk(Gap, 9 * t + c, [[18, W]]),
                           _mk(Gap, 9 * t + c + 6, [[18, W]]), Alu.subtract)
                        E[(t, c)] = e

                t9a = T9("t9a"); t9b = T9("t9b")
                # line-line: n = d1 x d2 per (i,j)
                nn = [T9(f"nn{c}") for c in range(3)]
                for c in range(3):
                    c1, c2 = (c + 1) % 3, (c + 2) % 3
                    tt(t9a, repA(E[(0, c1)]), repB(E[(1, c2)]), Alu.mult)
                    tt(t9b, repA(E[(0, c2)]), repB(E[(1, c1)]), Alu.mult)
                    tt(nn[c], t9a, t9b, Alu.subtract)
                r = [T9(f"r{c}") for c in range(3)]
                for c in range(3):
                    tt(s9(r[c]), R9A(c), R9B(c), Alu.subtract)
                rn = T9("rn"); dot3(rn, nn, r, t9a)
                n2 = T9("n2"); norm2(n2, nn, t9a)
                rn2 = T9("rn2")
                S.activation(out=rn2, in_=rn, func=Act.Square)
                fl9 = T9("fl9")
                V.scalar_tensor_tensor(out=fl9, in0=n2, scalar=M2, in1=rn2,
                                       op0=Alu.mult, op1=Alu.is_lt)
                V.scalar_tensor_tensor(out=fl9, in0=n2, scalar=N2MIN, in1=fl9,
                                       op0=Alu.is_gt, op1=Alu.mult)
                prune = TW("prune")
                V.memset(prune, 1.0)
                fold_and(fl9, 9, prune)

                # plane bounds per tri side
                for tp, tt_ in ((0, 1), (1, 0)):
                    eC0 = []
                    eC1 = []
                    for c in range(3):
                        x = TW(f"p1e0{c}")
                        tt(x, Cv(tt_, c, 1), Cv(tt_, c, 0), Alu.subtract)
                        eC0.append(x)
                        y = TW(f"p1e1{c}")
                        tt(y, Cv(tt_, c, 2), Cv(tt_, c, 0), Alu.subtract)
                        eC1.append(y)
                    nC = []
                    tw = TW("p1tw")
                    for c in range(3):
                        c1, c2 = (c + 1) % 3, (c + 2) % 3
                        z = TW(f"p1n{c}")
                        tt(tw, eC0[c1], eC1[c2], Alu.mult)
                        tt(z, eC0[c2], eC1[c1], Alu.mult)
                        tt(z, tw, z, Alu.subtract)
                        nC.append(z)
                    n2C = TW("p1n2"); norm2(n2C, nC, tw)
                    w3 = [T3(f"p1w{c}") for c in range(3)]
                    for c in range(3):
                        tt(s3(w3[c]), Rblk(tp, c), Cb3(tt_, c, 0), Alu.subtract)
                    wn = T3("p1wn"); dot3(wn, [b3(x) for x in nC], w3, T3("p1t3"))
                    wn2 = T3("p1wn2")
                    S.activation(out=wn2, in_=wn, func=Act.Square)
                    fl3 = T3("p1fl3")
                    V.scalar_tensor_tensor(out=fl3, in0=b3(n2C), scalar=M2, in1=s3(wn2),
                                           op0=Alu.mult, op1=Alu.is_lt)
                    V.scalar_tensor_tensor(out=fl3, in0=b3(n2C), scalar=N2MIN, in1=s3(fl3),
                                           op0=Alu.is_gt, op1=Alu.mult)
                    fold_and(fl3, 3, prune)

                nc.sync.dma_start(out=flags_out[:, colbase:colbase + W], in_=prune)
                colbase += W
    nc.compile()
    return nc


def kernel(triangles, close_idxs):
    triangles = np.ascontiguousarray(np.asarray(triangles, dtype=np.float32))
    ci = np.asarray(close_idxs)
    Bv, Pv = ci.shape[0], ci.shape[1]
    tbl = triangles.reshape(Bv * F, 9)

    recv_raw = ci[..., 0].reshape(-1)
    valid = recv_raw >= 0
    valid_count = max(float(valid.sum()), 1.0)

    ci32 = np.maximum(ci.astype(np.int64), 0).astype(np.int32)
    flat = ci32.reshape(-1, 2)
    batch_off = (np.arange(NPAIR, dtype=np.int64) // PPB * F).astype(np.int32)
    flat_abs = flat + batch_off[:, None]

    trace = bool(os.environ.get("BASS_KERNEL_TRACE"))
    tkw = dict(trace=trace, trace_cores=[0] if trace else None)
    exec_ns = 0

    if not os.environ.get("K_TWO_PHASE"):
        if "nc" not in _CACHE:
            _CACHE["nc"] = _build_kernel()
        nc = _CACHE["nc"]
        in_maps = []
        for c in range(NCORE):
            sl = flat_abs[c * PER_CORE:(c + 1) * PER_CORE]
            grid = np.zeros((CAP, 2), np.int32)
            grid[:PER_CORE] = sl
            mask = np.zeros(CAP, np.float32)
            mask[:PER_CORE] = valid[c * PER_CORE:(c + 1) * PER_CORE]
            gd = tbl[grid.reshape(-1)].reshape(CAP, 18).reshape(P, 18 * NCOL)
            in_maps.append({"gdata": gd, "maskin": mask.reshape(P, NCOL)})
        res = run_bass_kernel_spmd(nc, in_maps, list(range(NCORE)), **tkw)
        _CACHE["exec_time_ns"] = res.exec_time_ns
        total = sum(float(res.results[c]["psum"].astype(np.float64).sum())
                    for c in range(NCORE))
        return np.asarray(np.float32(total / valid_count))

    # ---------------- phase 1: certified prune over all pairs ----------------
    if "nc_flags" not in _CACHE:
        _CACHE["nc_flags"] = _build_flags()
    ncf = _CACHE["nc_flags"]
    CAP1 = P * NCOL1
    in_maps = []
    for c in range(NCORE):
        grid = np.zeros((CAP1, 2), np.int32)
        grid[:PER_CORE] = flat_abs[c * PER_CORE:(c + 1) * PER_CORE]
        gd = tbl[grid.reshape(-1)].reshape(CAP1, 18).reshape(P, 18 * NCOL1)
        in_maps.append({"gdata": gd})
    res1 = run_bass_kernel_spmd(ncf, in_maps, list(range(NCORE)), **tkw)
    if res1.exec_time_ns:
        exec_ns += res1.exec_time_ns

    surv = []
    for c in range(NCORE):
        fl = res1.results[c]["flags"].reshape(-1)[:PER_CORE]
        loc = np.nonzero((fl < 0.5) & valid[c * PER_CORE:(c + 1) * PER_CORE])[0]
        surv.append(loc + c * PER_CORE)
    surv = np.concatenate(surv)
    _CACHE["n_survivors"] = int(surv.size)

    if surv.size == 0:
        _CACHE["exec_time_ns"] = exec_ns
        return np.asarray(np.float32(0.0))

    # ---------------- phase 2: exact evaluation of survivors ----------------
    if "nc_p2" not in _CACHE:
        _CACHE["nc_p2"] = _build_kernel(ncol=CAP2_COL, tile_w=[CAP2_COL])
    nc2 = _CACHE["nc_p2"]
    rows = flat_abs[surv]                      # [S, 2]
    total = 0.0
    chunk = CAP2 * NCORE
    for s0 in range(0, surv.size, chunk):
        sub = rows[s0:s0 + chunk]
        n = sub.shape[0]
        in_maps = []
        for c in range(NCORE):
            grid = np.zeros((CAP2, 2), np.int32)
            mask = np.zeros(CAP2, np.float32)
            lo, hi = c * CAP2, min((c + 1) * CAP2, n)
            if hi > lo:
                grid[:hi - lo] = sub[lo:hi]
                mask[:hi - lo] = 1.0
            gd = tbl[grid.reshape(-1)].reshape(CAP2, 18).reshape(P, 18 * CAP2_COL)
            in_maps.append({"gdata": gd, "maskin": mask.reshape(P, CAP2_COL)})
        res2 = run_bass_kernel_spmd(nc2, in_maps, list(range(NCORE)), **tkw)
        if res2.exec_time_ns:
            exec_ns += res2.exec_time_ns
        total += sum(float(res2.results[c]["psum"].astype(np.float64).sum())
                     for c in range(NCORE))

    _CACHE["exec_time_ns"] = exec_ns if exec_ns else None
    return np.asarray(np.float32(total / valid_count))



# revision 8
# speedup vs baseline: 4.7009x; 4.7009x over previous
"""Trainium2 Bass kernel for nn_DistanceFieldPenetrationLoss.

loss = sum(relu(1e-3 - tridist(A,B))) / count over 2M close pairs, where
tridist is the reference's 15-term min (6 point/column-triangle distances +
9 row-edge/edge distances), data-parallel over 8 NeuronCores.

Two phases:
  1. fp16 certified prune over all pairs (planar SoA layout, DVE 2x mode).
     For each pair, accumulate relu-deficits of certified lower bounds:
     line-line distance |r.n|/|n| for the 9 edge pairs and point/plane
     distance for the 6 vertex/col-plane pairs, each tested against margin
     M=3.5e-3 (vs the 1e-3 threshold; the slack absorbs fp16 rounding).
     acc == 0 certifies pen == 0.  ~4% of pairs survive.
  2. fp32 exact evaluation of survivors (same 15-term algorithm as the
     reference), masked accumulate, host sums / divides by count.

Per-pair triangle rows are pre-gathered on the host (HW indirect-DMA
gathers are one-index-per-partition on TRN2) and streamed to SBUF.
"""
import os
import numpy as np

import concourse.bass as bass
import concourse.bacc as bacc
import concourse.mybir as mybir
import concourse.tile as tile
from concourse.bass_utils import run_bass_kernel_spmd

F32 = mybir.dt.float32
F16 = mybir.dt.float16
Alu = mybir.AluOpType
Act = mybir.ActivationFunctionType

P = 128
B, F, PPB = 4, 50000, 500000
NPAIR = B * PPB
NCORE = 8
PER_CORE = NPAIR // NCORE          # 250000
NCOL1 = 1954                       # 128*1954 = 250112 slots per core
CAP1 = P * NCOL1
TILE_W1 = [512, 512, 512, 418]

EPS = 1e-12
LOSS_EPS = 1e-3
BIG = 1e30
M2 = float(3.5e-3) ** 2            # phase-1 certified margin, squared
N2MIN = 5e-3                       # min |cross|^2 for trustworthy normal
SCLN = 0.35                        # sqrt(kappa)*M scale for normal squares
SCLR = 100.0                       # sqrt(kappa) scale for rn/wn squares
GTHR = 0.1225 * N2MIN              # guard threshold on scaled n2

CAP2_COL = 104                     # phase-2 slots per core = 13312
CAP2 = P * CAP2_COL

_CACHE = {}


def _mk(ap, off, dims):
    """View into an SBUF tile AP with explicit free dims [[step, count], ...]."""
    return bass.AP(ap.tensor, ap.offset + off, [list(ap.ap[0])] + [list(d) for d in dims])


# ---------------------------------------------------------------------------
# Phase 1: fp16 certified prune.  gdata planar: plane m = 9t + (3i + c) for
# row-vertex i comp c of tri t; pair (p, col) at [p, m*NCOL1 + col].
# Output acc[p, col] = sum of relu deficits; 0 <=> certified pen == 0.
# ---------------------------------------------------------------------------

def _build_cert16():
    nc = bacc.Bacc("TRN2", target_bir_lowering=False, debug=False)
    g = nc.declare_dram_parameter("g16", [P, 18 * NCOL1], F16, isOutput=False)
    acc_out = nc.declare_dram_parameter("acc", [P, NCOL1], F16, isOutput=True)

    with tile.TileContext(nc) as tc:
        with (
            tc.tile_pool(name="gio", bufs=2) as gio,
            tc.tile_pool(name="work", bufs=1) as work,
        ):
            V = nc.vector
            S = nc.scalar
            GP = nc.gpsimd

            colbase = 0
            for ti, W in enumerate(TILE_W1):
                # tile ti is stored planar-within-tile: plane m of this tile
                # occupies gdata columns [18*colbase + m*W, ... + (m+1)*W)
                G = gio.tile([P, 18 * W], F16, tag="g", name="g")
                nc.sync.dma_start(out=G[:], in_=g[:, 18 * colbase:18 * (colbase + W)])
                Gap = G[:]

                def PL(m, blocks=None):
                    """Plane view; blocks = [[step_planes, count], ...] outer dims."""
                    if blocks is None:
                        return _mk(Gap, m * W, [[1, W]])
                    dims = [[s * W, n] for (s, n) in blocks] + [[1, W]]
                    return _mk(Gap, m * W, dims)

                def TT(tag, units):
                    return work.tile([P, units * W], F16, tag=tag, name=tag)[:]

                acc = TT("acc", 1)
                V.memset(acc, 0.0)

                # ---- row edges E[t][c]: [3(edge i), W]; e2 = v0 - v2 ----
                E = {}
                for t in (0, 1):
                    for c in range(3):
                        e = TT(f"E{t}{c}", 3)
                        V.tensor_tensor(
                            out=_mk(e, 0, [[W, 2], [1, W]]),
                            in0=PL(9 * t + 3 + c, [[3, 2]]),
                            in1=PL(9 * t + c, [[3, 2]]),
                            op=Alu.subtract)
                        V.tensor_tensor(
                            out=_mk(e, 2 * W, [[1, W]]),
                            in0=PL(9 * t + c),
                            in1=PL(9 * t + 6 + c),
                            op=Alu.subtract)
                        E[(t, c)] = e

                # ---- cross products N[c][(i,j)] = +-(eA_i x eB_j)_c ----
                # direct for (i,j) in {0,1}^2, derived rows/cols by sums
                # (signs differ per block; only squares/products-with-r used).
                N = [TT(f"N{c}", 9) for c in range(3)]
                q1 = TT("q1", 9)
                q2 = TT("q2", 9)
                v22 = [[3 * W, 2], [W, 2], [1, W]]
                for c in range(3):
                    c1, c2 = (c + 1) % 3, (c + 2) % 3
                    eA1 = _mk(E[(0, c1)], 0, [[W, 2], [0, 2], [1, W]])
                    eB2 = _mk(E[(1, c2)], 0, [[0, 2], [W, 2], [1, W]])
                    eA2 = _mk(E[(0, c2)], 0, [[W, 2], [0, 2], [1, W]])
                    eB1 = _mk(E[(1, c1)], 0, [[0, 2], [W, 2], [1, W]])
                    V.tensor_tensor(out=_mk(q1, 0, v22), in0=eA1, in1=eB2, op=Alu.mult)
                    V.tensor_tensor(out=_mk(q2, 0, v22), in0=eA2, in1=eB1, op=Alu.mult)
                    V.tensor_tensor(out=_mk(N[c], 0, v22), in0=_mk(q1, 0, v22),
                                    in1=_mk(q2, 0, v22), op=Alu.subtract)
                    # row i=2 (blocks 6,7) = block0 + block3
                    V.tensor_tensor(out=_mk(N[c], 6 * W, [[W, 2], [1, W]]),
                                    in0=_mk(N[c], 0, [[W, 2], [1, W]]),
                                    in1=_mk(N[c], 3 * W, [[W, 2], [1, W]]),
                                    op=Alu.add)
                    # col j=2 (blocks 2,5,8) = col0 + col1
                    V.tensor_tensor(out=_mk(N[c], 2 * W, [[3 * W, 3], [1, W]]),
                                    in0=_mk(N[c], 0, [[3 * W, 3], [1, W]]),
                                    in1=_mk(N[c], 1 * W, [[3 * W, 3], [1, W]]),
                                    op=Alu.add)

                # ---- r[c][(i,j)] = A_i^c - B_j^c ----
                R = [TT(f"R{c}", 9) for c in range(3)]
                for c in range(3):
                    V.tensor_tensor(
                        out=R[c],
                        in0=PL(c, [[3, 3], [0, 3]]),
                        in1=PL(9 + c, [[0, 3], [3, 3]]),
                        op=Alu.subtract)

                # ---- rn = r . n ; n2 = |n|^2 (9-blocked) ----
                rn = TT("rn", 9)
                V.tensor_tensor(out=q1, in0=R[0], in1=N[0], op=Alu.mult)
                V.tensor_tensor(out=rn, in0=R[1], in1=N[1], op=Alu.mult)
                V.tensor_tensor(out=rn, in0=rn, in1=q1, op=Alu.add)
                V.tensor_tensor(out=q1, in0=R[2], in1=N[2], op=Alu.mult)
                V.tensor_tensor(out=rn, in0=rn, in1=q1, op=Alu.add)
                # scaled n2s = kappa*M^2*|n|^2 via Square scale
                n2 = TT("n2", 9)
                S.activation(out=q1, in_=N[0], func=Act.Square, scale=SCLN)
                S.activation(out=n2, in_=N[1], func=Act.Square, scale=SCLN)
                V.tensor_tensor(out=n2, in0=n2, in1=q1, op=Alu.add)
                S.activation(out=q1, in_=N[2], func=Act.Square, scale=SCLN)
                V.tensor_tensor(out=n2, in0=n2, in1=q1, op=Alu.add)
                # rn2s = kappa*rn^2 in q2
                S.activation(out=q2, in_=rn, func=Act.Square, scale=SCLR)
                # deficit relu(n2s - rn2s); guard relu(1 - n2s/GTHR)
                V.tensor_tensor(out=q1, in0=n2, in1=q2, op=Alu.subtract)
                S.activation(out=q1, in_=q1, func=Act.Relu)
                S.activation(out=q2, in_=n2, func=Act.Relu, scale=-1.0 / GTHR,
                             bias=1.0)
                V.tensor_tensor(out=q1, in0=q1, in1=q2, op=Alu.add)
                # fold 9 -> acc (9 -> 5 -> 3 -> 2 -> 1; every block covered)
                V.tensor_tensor(out=_mk(q1, 0, [[1, 4 * W]]),
                                in0=_mk(q1, 0, [[1, 4 * W]]),
                                in1=_mk(q1, 5 * W, [[1, 4 * W]]), op=Alu.add)
                V.tensor_tensor(out=_mk(q1, 0, [[1, 2 * W]]),
                                in0=_mk(q1, 0, [[1, 2 * W]]),
                                in1=_mk(q1, 3 * W, [[1, 2 * W]]), op=Alu.add)
                V.tensor_tensor(out=_mk(q1, 0, [[1, W]]),
                                in0=_mk(q1, 0, [[1, W]]),
                                in1=_mk(q1, 2 * W, [[1, W]]), op=Alu.add)
                V.tensor_tensor(out=_mk(q1, 0, [[1, W]]),
                                in0=_mk(q1, 0, [[1, W]]),
                                in1=_mk(q1, 1 * W, [[1, W]]), op=Alu.add)
                V.tensor_tensor(out=acc, in0=acc, in1=_mk(q1, 0, [[1, W]]), op=Alu.add)

                # ---- plane tests: rows of tp vs column-plane of tt ----
                # col-vertex k comp c of tri t: plane 9t + 3c + k
                eC0 = TT("eC0", 3)
                eC1 = TT("eC1", 3)
                nC = TT("nC", 3)
                t3 = TT("t3", 3)
                w3 = TT("w3", 3)
                wn = TT("wn", 3)
                n2C = TT("n2C", 1)
                gW = TT("gW", 1)
                for tp, tt_ in ((0, 1), (1, 0)):
                    V.tensor_tensor(out=eC0, in0=PL(9 * tt_ + 1, [[3, 3]]),
                                    in1=PL(9 * tt_, [[3, 3]]), op=Alu.subtract)
                    V.tensor_tensor(out=eC1, in0=PL(9 * tt_ + 2, [[3, 3]]),
                                    in1=PL(9 * tt_, [[3, 3]]), op=Alu.subtract)
                    for c in range(3):
                        c1, c2 = (c + 1) % 3, (c + 2) % 3
                        V.tensor_tensor(out=_mk(t3, 0, [[1, W]]),
                                        in0=_mk(eC0, c1 * W, [[1, W]]),
                                        in1=_mk(eC1, c2 * W, [[1, W]]), op=Alu.mult)
                        V.tensor_tensor(out=_mk(t3, W, [[1, W]]),
                                        in0=_mk(eC0, c2 * W, [[1, W]]),
                                        in1=_mk(eC1, c1 * W, [[1, W]]), op=Alu.mult)
                        V.tensor_tensor(out=_mk(nC, c * W, [[1, W]]),
                                        in0=_mk(t3, 0, [[1, W]]),
                                        in1=_mk(t3, W, [[1, W]]), op=Alu.subtract)
                    S.activation(out=t3, in_=nC, func=Act.Square, scale=SCLN)
                    V.tensor_tensor(out=n2C, in0=_mk(t3, 0, [[1, W]]),
                                    in1=_mk(t3, W, [[1, W]]), op=Alu.add)
                    V.tensor_tensor(out=n2C, in0=n2C,
                                    in1=_mk(t3, 2 * W, [[1, W]]), op=Alu.add)
                    S.activation(out=gW, in_=n2C, func=Act.Relu,
                                 scale=-1.0 / GTHR, bias=1.0)
                    # wn = sum_c (rowA_c - colB0_c) * nC_c   [3(vertex i), W]
                    for c in range(3):
                        V.tensor_tensor(out=w3,
                                        in0=PL(9 * tp + c, [[3, 3]]),
                                        in1=PL(9 * tt_ + 3 * c, [[0, 3]]),
                                        op=Alu.subtract)
                        if c == 0:
                            V.tensor_tensor(out=wn, in0=w3,
                                            in1=_mk(nC, 0, [[0, 3], [1, W]]),
                                            op=Alu.mult)
                        else:
                            V.tensor_tensor(out=w3, in0=w3,
                                            in1=_mk(nC, c * W, [[0, 3], [1, W]]),
                                            op=Alu.mult)
                            V.tensor_tensor(out=wn, in0=wn, in1=w3, op=Alu.add)
                    S.activation(out=t3, in_=wn, func=Act.Square, scale=SCLR)
                    # deficit relu(M2*n2C - wn^2) per vertex
                    V.tensor_tensor(out=t3, in0=_mk(n2C, 0, [[0, 3], [1, W]]),
                                    in1=t3, op=Alu.subtract)
                    S.activation(out=t3, in_=t3, func=Act.Relu)
                    V.tensor_tensor(out=_mk(t3, 0, [[1, W]]),
                                    in0=_mk(t3, 0, [[1, W]]),
                                    in1=_mk(t3, 2 * W, [[1, W]]), op=Alu.add)
                    V.tensor_tensor(out=_mk(t3, 0, [[1, W]]),
                                    in0=_mk(t3, 0, [[1, W]]),
                                    in1=_mk(t3, 1 * W, [[1, W]]), op=Alu.add)
                    V.tensor_tensor(out=_mk(t3, 0, [[1, W]]),
                                    in0=_mk(t3, 0, [[1, W]]), in1=gW, op=Alu.add)
                    V.tensor_tensor(out=acc, in0=acc,
                                    in1=_mk(t3, 0, [[1, W]]), op=Alu.add)

                nc.sync.dma_start(out=acc_out[:, colbase:colbase + W], in_=acc)
                colbase += W
    nc.compile()
    return nc


# ---------------------------------------------------------------------------
# Phase 2: fp32 exact evaluation (reference-faithful 15-term min).
# gdata interleaved: pair (p, col) occupies free [col*18, col*18+18).
# ---------------------------------------------------------------------------

def _build_exact(ncol, tile_w):
    NCOL = ncol
    TILE_W = tile_w
    nc = bacc.Bacc("TRN2", target_bir_lowering=False, debug=False)
    gdata = nc.declare_dram_parameter("gdata", [P, 18 * NCOL], F32, isOutput=False)
    maskin = nc.declare_dram_parameter("maskin", [P, NCOL], F32, isOutput=False)
    psum_out = nc.declare_dram_parameter("psum", [P, len(TILE_W)], F32, isOutput=True)

    with tile.TileContext(nc) as tc:
        with (
            tc.tile_pool(name="gio", bufs=2) as gio,
            tc.tile_pool(name="work", bufs=1) as work,
        ):
            V = nc.vector
            S = nc.scalar
            GP = nc.gpsimd

            psum_t = work.tile([P, len(TILE_W)], F32, tag="psum", name="psum")
            V.memset(psum_t[:], 0.0)

            colbase = 0
            for ti, W in enumerate(TILE_W):
                G = gio.tile([P, 18 * W], F32, tag="g", name="g")
                nc.sync.dma_start(out=G[:], in_=gdata[:, 18 * colbase:18 * (colbase + W)])
                M = gio.tile([P, W], F32, tag="mask", name="mask")
                nc.sync.dma_start(out=M[:], in_=maskin[:, colbase:colbase + W])

                Gap = G[:]

                def Rblk(t, c):          # [3(vert i), W]
                    return _mk(Gap, 9 * t + c, [[3, 3], [18, W]])

                def R9A(c):              # [3(i), 3(rep j), W]
                    return _mk(Gap, c, [[3, 3], [0, 3], [18, W]])

                def R9B(c):              # [3(rep i), 3(j), W]
                    return _mk(Gap, 9 + c, [[0, 3], [3, 3], [18, W]])

                def Cv(t, c, k):         # [W] single col-vertex comp
                    return _mk(Gap, 9 * t + 3 * c + k, [[18, W]])

                def Cb3(t, c, k):        # [3(rep), W]
                    return _mk(Gap, 9 * t + 3 * c + k, [[0, 3], [18, W]])

                def TW(tag):
                    return work.tile([P, W], F32, tag=tag, name=tag)[:]

                def T3(tag):
                    return work.tile([P, 3 * W], F32, tag=tag, name=tag)[:]

                def T9(tag):
                    return work.tile([P, 9 * W], F32, tag=tag, name=tag)[:]

                def s3(ap):              # [3, W] view of 3W tile
                    return _mk(ap, 0, [[W, 3], [1, W]])

                def s9(ap):              # [3, 3, W] view of 9W tile
                    return _mk(ap, 0, [[3 * W, 3], [W, 3], [1, W]])

                def b3(ap_w):            # broadcast [W] tile over 3 blocks
                    return _mk(ap_w, 0, [[0, 3], [1, W]])

                def repA(ap3):           # [3W] tile -> [3(i), 3(rep), W]
                    return _mk(ap3, 0, [[W, 3], [0, 3], [1, W]])

                def repB(ap3):           # [3W] tile -> [3(rep), 3(j), W]
                    return _mk(ap3, 0, [[0, 3], [W, 3], [1, W]])

                def blkof(ap3, i):       # i-th W block of a 3W tile
                    return _mk(ap3, i * W, [[1, W]])

                def tt(out, a, b, op, eng=None):
                    (eng or V).tensor_tensor(out=out, in0=a, in1=b, op=op)

                def dot3(out, av, bv, tmp, addeng=None):
                    tt(tmp, av[0], bv[0], Alu.mult)
                    tt(out, av[1], bv[1], Alu.mult)
                    tt(out, out, tmp, Alu.add)
                    tt(tmp, av[2], bv[2], Alu.mult)
                    tt(out, out, tmp, Alu.add)

                def norm2(out, av, tmp):
                    S.activation(out=tmp, in_=av[0], func=Act.Square)
                    S.activation(out=out, in_=av[1], func=Act.Square)
                    tt(out, out, tmp, Alu.add)
                    S.activation(out=tmp, in_=av[2], func=Act.Square)
                    tt(out, out, tmp, Alu.add)

                def recip(out, x, tmp):
                    V.reciprocal_approx_accurate(out=out, in_=x, scratch=tmp)

                def clip01(x):
                    V.tensor_scalar(out=x, in0=x, scalar1=0.0, scalar2=1.0,
                                    op0=Alu.max, op1=Alu.min)

                acc = TW("acc")
                V.memset(acc, BIG)

                def foldmin(blocked_ap_tile, nblk):
                    n = nblk
                    while n > 1:
                        h = n // 2
                        lo = _mk(blocked_ap_tile, 0, [[1, h * W]])
                        hi = _mk(blocked_ap_tile, (n - h) * W, [[1, h * W]])
                        tt(lo, lo, hi, Alu.min)
                        n = n - h
                    tt(acc, acc, _mk(blocked_ap_tile, 0, [[1, W]]), Alu.min)

                # ---------- per-side derived data ----------
                side = []
                for t in (0, 1):
                    sd = {}
                    eC = []
                    for pair_kk in ((1, 0), (2, 0), (2, 1)):
                        comps = []
                        for c in range(3):
                            e = TW(f"eC{len(eC)}{c}_{t}")
                            tt(e, Cv(t, c, pair_kk[0]), Cv(t, c, pair_kk[1]), Alu.subtract)
                            comps.append(e)
                        eC.append(comps)
                    sd["eC"] = eC
                    tmpw = TW(f"tmpw{t}")
                    aC = TW(f"aC{t}"); norm2(aC, eC[0], tmpw)
                    V.tensor_scalar(out=aC, in0=aC, scalar1=EPS, scalar2=None, op0=Alu.max)
                    bC = TW(f"bC{t}"); dot3(bC, eC[0], eC[1], tmpw)
                    cC = TW(f"cC{t}"); norm2(cC, eC[1], tmpw)
                    V.tensor_scalar(out=cC, in0=cC, scalar1=EPS, scalar2=None, op0=Alu.max)
                    a2C = TW(f"a2C{t}"); norm2(a2C, eC[2], tmpw)
                    V.tensor_scalar(out=a2C, in0=a2C, scalar1=EPS, scalar2=None, op0=Alu.max)
                    det = TW(f"det{t}")
                    S.activation(out=tmpw, in_=bC, func=Act.Square)
                    tt(det, aC, cC, Alu.mult)
                    tt(det, det, tmpw, Alu.subtract)
                    V.tensor_scalar(out=det, in0=det, scalar1=EPS, scalar2=None, op0=Alu.max)
                    for nm, src in (("invdet", det), ("invaC", aC), ("invcC", cC), ("inva2C", a2C)):
                        dst = TW(nm + str(t)); recip(dst, src, tmpw)
                        sd[nm] = dst
                    sd.update(aC=aC, bC=bC, cC=cC, a2C=a2C, det=det)

                    E = []
                    for c in range(3):
                        e = T3(f"E{c}_{t}")
                        nxt = _mk(Gap, 9 * t + c + 3, [[3, 2], [18, W]])
                        cur = _mk(Gap, 9 * t + c, [[3, 2], [18, W]])
                        tt(_mk(e, 0, [[W, 2], [1, W]]), nxt, cur, Alu.subtract)
                        tt(blkof(e, 2), _mk(Gap, 9 * t + c, [[18, W]]),
                           _mk(Gap, 9 * t + c + 6, [[18, W]]), Alu.subtract)
                        E.append(e)
                    sd["E"] = E
                    tmp3 = T3(f"tmp3_{t}")
                    aE = T3(f"aE{t}")
                    norm2(aE, E, tmp3)
                    invE = T3(f"invE{t}"); recip(invE, aE, tmp3)
                    ninvE = T3(f"ninvE{t}")
                    V.tensor_scalar(out=ninvE, in0=invE, scalar1=-1.0, scalar2=None, op0=Alu.mult)
                    sd.update(aE=aE, invE=invE, ninvE=ninvE)
                    side.append(sd)

                # ---------- point-triangle, 2 directions x 3 points ----------
                t3a = T3("t3a"); t3b = T3("t3b"); t3c = T3("t3c")
                for tp, tt_ in ((0, 1), (1, 0)):
                    sd = side[tt_]
                    w = [T3(f"w{c}") for c in range(3)]
                    for c in range(3):
                        tt(s3(w[c]), Rblk(tp, c), Cb3(tt_, c, 0), Alu.subtract)
                    d = T3("ptd"); dot3(d, [b3(x) for x in sd["eC"][0]], w, t3a, GP)
                    e = T3("pte"); dot3(e, [b3(x) for x in sd["eC"][1]], w, t3a, GP)
                    f = T3("ptf"); norm2(f, w, t3a)
                    s = T3("pts")
                    tt(t3a, b3(sd["bC"]), e, Alu.mult)
                    tt(t3b, b3(sd["cC"]), d, Alu.mult)
                    tt(s, t3a, t3b, Alu.subtract)
                    t = T3("ptt")
                    tt(t3a, b3(sd["bC"]), d, Alu.mult)
                    tt(t3b, b3(sd["aC"]), e, Alu.mult)
                    tt(t, t3a, t3b, Alu.subtract)
                    m = T3("ptm")
                    tt(m, s, t, Alu.min)
                    tt(t3a, s, t, Alu.add)
                    V.scalar_tensor_tensor(out=t3b, in0=t3a, scalar=-1.0, in1=b3(sd["det"]),
                                           op0=Alu.mult, op1=Alu.add)
                    tt(m, m, t3b, Alu.min)
                    fc = T3("ptfc")
                    tt(t3a, d, s, Alu.mult)
                    tt(t3b, e, t, Alu.mult)
                    tt(t3a, t3a, t3b, Alu.add)
                    tt(t3b, f, b3(sd["det"]), Alu.mult)
                    tt(t3a, t3b, t3a, Alu.subtract)
                    tt(fc, t3a, b3(sd["invdet"]), Alu.mult)
                    V.tensor_scalar(out=fc, in0=fc, scalar1=0.0, scalar2=None, op0=Alu.max)
                    V.tensor_scalar(out=t3a, in0=m, scalar1=0.0, scalar2=BIG,
                                    op0=Alu.is_lt, op1=Alu.mult)
                    tt(fc, fc, t3a, Alu.add)
                    foldmin(fc, 3)
                    for dotv, inv, ev in ((d, "invaC", 0), (e, "invcC", 1)):
                        u = t3c
                        tt(u, dotv, b3(sd[inv]), Alu.mult)
                        clip01(u)
                        pe = T3("ptpe")
                        for c in range(3):
                            tt(t3a, u, b3(sd["eC"][ev][c]), Alu.mult)
                            tt(t3b, w[c], t3a, Alu.subtract)
                            if c == 0:
                                S.activation(out=pe, in_=t3b, func=Act.Square)
                            else:
                                S.activation(out=t3a, in_=t3b, func=Act.Square)
                                tt(pe, pe, t3a, Alu.add)
                        foldmin(pe, 3)
                    w2 = [T3(f"w2{c}") for c in range(3)]
                    for c in range(3):
                        tt(s3(w2[c]), Rblk(tp, c), Cb3(tt_, c, 1), Alu.subtract)
                    dd = T3("ptdd"); dot3(dd, [b3(x) for x in sd["eC"][2]], w2, t3a, GP)
                    u = t3c
                    tt(u, dd, b3(sd["inva2C"]), Alu.mult)
                    clip01(u)
                    pe = T3("ptpe")
                    for c in range(3):
                        tt(t3a, u, b3(sd["eC"][2][c]), Alu.mult)
                        tt(t3b, w2[c], t3a, Alu.subtract)
                        if c == 0:
                            S.activation(out=pe, in_=t3b, func=Act.Square)
                        else:
                            S.activation(out=t3a, in_=t3b, func=Act.Square)
                            tt(pe, pe, t3a, Alu.add)
                    foldmin(pe, 3)

                # ---------- edge-edge, 9-blocked ----------
                EA, EB = side[0]["E"], side[1]["E"]
                t9a = T9("t9a"); t9b = T9("t9b"); t9c = T9("t9c")
                r = [T9(f"r{c}") for c in range(3)]
                for c in range(3):
                    tt(s9(r[c]), R9A(c), R9B(c), Alu.subtract)
                d1v = [repA(EA[c]) for c in range(3)]
                d2v = [repB(EB[c]) for c in range(3)]
                cdot = T9("cdot"); dot3(cdot, d1v, r, t9a, GP)
                fdot = T9("fdot"); dot3(fdot, d2v, r, t9a, GP)
                bq = T9("bq"); dot3(bq, d1v, d2v, t9a, GP)
                aA_r = repA(side[0]["aE"]); aB_t = repB(side[1]["aE"])
                den = T9("den")
                tt(den, aA_r, aB_t, Alu.mult)
                S.activation(out=t9a, in_=bq, func=Act.Square)
                tt(den, den, t9a, Alu.subtract)
                dadj = T9("dadj")
                V.tensor_scalar(out=dadj, in0=den, scalar1=EPS, scalar2=None, op0=Alu.max)
                invd = T9("invd")
                recip(invd, dadj, t9a)
                s = T9("ees")
                tt(t9a, bq, fdot, Alu.mult)
                tt(t9b, cdot, aB_t, Alu.mult)
                tt(s, t9a, t9b, Alu.subtract)
                tt(s, s, invd, Alu.mult)
                t = T9("eet")
                tt(t9a, aA_r, fdot, Alu.mult)
                tt(t9b, bq, cdot, Alu.mult)
                tt(t, t9a, t9b, Alu.subtract)
                tt(t, t, invd, Alu.mult)
                # Clamped interior form; boundary terms below realize the rest.
                clip01(s)
                clip01(t)
                d2i = T9("d2i")
                for c in range(3):
                    tt(t9a, s, d1v[c], Alu.mult)
                    tt(t9a, r[c], t9a, Alu.add)
                    tt(t9b, t, d2v[c], Alu.mult)
                    tt(t9a, t9a, t9b, Alu.subtract)
                    if c == 0:
                        S.activation(out=d2i, in_=t9a, func=Act.Square)
                    else:
                        S.activation(out=t9b, in_=t9a, func=Act.Square)
                        tt(d2i, d2i, t9b, Alu.add)
                foldmin(d2i, 9)
                u = t9c
                tt(u, fdot, repB(side[1]["invE"]), Alu.mult)
                clip01(u)
                pe9 = T9("pe9")
                for c in range(3):
                    tt(t9a, u, d2v[c], Alu.mult)
                    tt(t9a, r[c], t9a, Alu.subtract)
                    if c == 0:
                        S.activation(out=pe9, in_=t9a, func=Act.Square)
                    else:
                        S.activation(out=t9b, in_=t9a, func=Act.Square)
                        tt(pe9, pe9, t9b, Alu.add)
                foldmin(pe9, 9)
                tt(u, cdot, repA(side[0]["ninvE"]), Alu.mult)
                clip01(u)
                for c in range(3):
                    tt(t9a, u, d1v[c], Alu.mult)
                    tt(t9a, r[c], t9a, Alu.add)
                    if c == 0:
                        S.activation(out=pe9, in_=t9a, func=Act.Square)
                    else:
                        S.activation(out=t9b, in_=t9a, func=Act.Square)
                        tt(pe9, pe9, t9b, Alu.add)
                foldmin(pe9, 9)

                # ---------- pen = relu(1e-3 - sqrt(acc)); masked accumulate ----------
                dist = TW("dist")
                S.activation(out=dist, in_=acc, func=Act.Sqrt)
                pen = TW("pen")
                V.tensor_scalar(out=pen, in0=dist, scalar1=-1.0, scalar2=LOSS_EPS,
                                op0=Alu.mult, op1=Alu.add)
                penm = TW("penm")
                V.scalar_tensor_tensor(out=penm, in0=pen, scalar=0.0, in1=M[:],
                                       op0=Alu.max, op1=Alu.mult,
                                       accum_out=psum_t[:, ti:ti + 1])
                colbase += W

            nc.sync.dma_start(out=psum_out[:], in_=psum_t[:])
    nc.compile()
    return nc


# ---------------------------------------------------------------------------
# Host driver
# ---------------------------------------------------------------------------

def kernel(triangles, close_idxs):
    triangles = np.ascontiguousarray(np.asarray(triangles, dtype=np.float32))
    ci = np.asarray(close_idxs)
    tbl = triangles.reshape(B * F, 9)
    tbl16 = tbl.astype(np.float16)

    recv_raw = ci[..., 0].reshape(-1)
    valid = recv_raw >= 0
    valid_count = max(float(valid.sum()), 1.0)

    ci32 = np.maximum(ci.astype(np.int64), 0).astype(np.int32)
    flat = ci32.reshape(-1, 2)
    batch_off = (np.arange(NPAIR, dtype=np.int64) // PPB * F).astype(np.int32)
    flat_abs = flat + batch_off[:, None]

    trace = bool(os.environ.get("BASS_KERNEL_TRACE"))
    tkw = dict(trace=trace, trace_cores=[0] if trace else None)
    exec_ns = 0

    # ---------------- phase 1: fp16 certified prune ----------------
    if "nc_cert" not in _CACHE:
        _CACHE["nc_cert"] = _build_cert16()
    ncc = _CACHE["nc_cert"]
    in_maps = []
    for c in range(NCORE):
        grid = np.zeros((CAP1, 2), np.int32)
        grid[:PER_CORE] = flat_abs[c * PER_CORE:(c + 1) * PER_CORE]
        gd = tbl16[grid.reshape(-1)].reshape(P, NCOL1, 18)
        # planar-within-tile layout: each DMA tile is a contiguous [18, W] block
        parts = []
        cb = 0
        for Wt in TILE_W1:
            parts.append(np.ascontiguousarray(
                gd[:, cb:cb + Wt, :].transpose(0, 2, 1)).reshape(P, 18 * Wt))
            cb += Wt
        in_maps.append({"g16": np.concatenate(parts, axis=1)})
    res1 = run_bass_kernel_spmd(ncc, in_maps, list(range(NCORE)), **tkw)
    if res1.exec_time_ns:
        exec_ns += res1.exec_time_ns

    surv = []
    for c in range(NCORE):
        accv = res1.results[c]["acc"].reshape(-1)[:PER_CORE]
        loc = np.nonzero((accv > 0) & valid[c * PER_CORE:(c + 1) * PER_CORE])[0]
        surv.append(loc + c * PER_CORE)
    surv = np.concatenate(surv)
    _CACHE["n_survivors"] = int(surv.size)

    if surv.size == 0:
        _CACHE["exec_time_ns"] = exec_ns if exec_ns else None
        return np.asarray(np.float32(0.0))

    # ---------------- phase 2: exact evaluation of survivors ----------------
    if "nc_p2" not in _CACHE:
        _CACHE["nc_p2"] = _build_exact(CAP2_COL, [CAP2_COL])
    nc2 = _CACHE["nc_p2"]
    rows = flat_abs[surv]
    total = 0.0
    chunk = CAP2 * NCORE
    for s0 in range(0, surv.size, chunk):
        sub = rows[s0:s0 + chunk]
        n = sub.shape[0]
        in_maps = []
        for c in range(NCORE):
            grid = np.zeros((CAP2, 2), np.int32)
            mask = np.zeros(CAP2, np.float32)
            lo, hi = c * CAP2, min((c + 1) * CAP2, n)
            if hi > lo:
                grid[:hi - lo] = sub[lo:hi]
                mask[:hi - lo] = 1.0
            gd = tbl[grid.reshape(-1)].reshape(CAP2, 18).reshape(P, 18 * CAP2_COL)
            in_maps.append({"gdata": gd, "maskin": mask.reshape(P, CAP2_COL)})
        res2 = run_bass_kernel_spmd(nc2, in_maps, list(range(NCORE)), **tkw)
        if res2.exec_time_ns:
            exec_ns += res2.exec_time_ns
        total += sum(float(res2.results[c]["psum"].astype(np.float64).sum())
                     for c in range(NCORE))

    _CACHE["exec_time_ns"] = exec_ns if exec_ns else None
    return np.asarray(np.float32(total / valid_count))
